# revision 29
# baseline (speedup 1.0000x reference)
"""Sliding-window GQA attention on 8 TRN2 NeuronCores, tensor-parallel by heads.

Core c owns KV head c and Q heads 4c..4c+3.  All device matmuls run in bf16.
Structure: 4 sequence chunks of 512; per chunk QKV projection + RoPE, then
windowed attention (scores transposed [k,q], exp on ACT, post-exp 0/1 masks on
DVE), pv with a ones-column denominator, per-partition normalize, PE transpose
to [dh,s], then the wo out-projection.  Chunks pipeline: attention of chunk i
overlaps QKV of chunk i+1 on complementary engines.  Each core emits a partial
output (wo input-dim sharded); the host sums the 8 partials.
"""

import os
import sys

sys.path.insert(0, "/opt/trn_rl_repo")

import numpy as np
import ml_dtypes

SEQ = 2048
DIM = 4096
N_HEADS = 32
N_KV = 8
HD = 128
WIN = 1024
NCORES = 8
QH = N_HEADS // N_KV          # 4 q heads per core
DHL = QH * HD                 # 512 local q dims
P = 128
DB = DIM // P                 # 32 contraction blocks
SC = 512                      # seq chunk
NSC = SEQ // SC               # 4 chunks
BPC = SC // P                 # 4 i-blocks per chunk
NIB = SEQ // P                # 16 blocks total
WB = WIN // P                 # 8 window blocks

BF = ml_dtypes.bfloat16


def _build_nc():
    import concourse.mybir as mybir
    from concourse import bacc
    from concourse.tile import TileContext

    f32 = mybir.dt.float32
    bf = mybir.dt.bfloat16

    nc = bacc.Bacc()
    xt = nc.declare_dram_parameter("xt", [DIM, SEQ], bf, isOutput=False)
    wqt = nc.declare_dram_parameter("wqt", [DIM, DHL], bf, isOutput=False)
    wkt = nc.declare_dram_parameter("wkt", [DIM, HD], bf, isOutput=False)
    wvt = nc.declare_dram_parameter("wvt", [DIM, HD], bf, isOutput=False)
    wot = nc.declare_dram_parameter("wot", [DHL, DIM], bf, isOutput=False)
    cexp = nc.declare_dram_parameter("cexp", [P, SEQ], bf, isOutput=False)
    sexp = nc.declare_dram_parameter("sexp", [P, SEQ], bf, isOutput=False)
    rt = nc.declare_dram_parameter("rt", [P, P], bf, isOutput=False)
    ident = nc.declare_dram_parameter("ident", [P, P], bf, isOutput=False)
    md01 = nc.declare_dram_parameter("md01", [P, P], bf, isOutput=False)
    mt01 = nc.declare_dram_parameter("mt01", [P, P], bf, isOutput=False)
    out = nc.declare_dram_parameter("out", [SEQ, DIM], bf, isOutput=True)

    Exp = mybir.ActivationFunctionType.Exp

    with TileContext(nc) as tc:
        with (
            tc.tile_pool(name="const", bufs=1) as cp,
            tc.tile_pool(name="wp", bufs=1) as wp,
            tc.tile_pool(name="kvp", bufs=1) as kvp,
            tc.tile_pool(name="xtp", bufs=1) as xtp,
            tc.tile_pool(name="qrp", bufs=2) as qrp,
            tc.tile_pool(name="atp", bufs=2) as atp,
            tc.tile_pool(name="t12", bufs=2) as t12p,
            tc.tile_pool(name="expt", bufs=13) as etp,
            tc.tile_pool(name="asp", bufs=5) as asp,
            tc.tile_pool(name="osb", bufs=2) as osbp,
            tc.tile_pool(name="psb", bufs=2, space="PSUM") as psbig,
            tc.tile_pool(name="pss", bufs=2, space="PSUM") as pssc,
            tc.tile_pool(name="pvt", bufs=2, space="PSUM") as pspv,
            tc.tile_pool(name="pso", bufs=2, space="PSUM") as psop,
        ):
            xt_r = xt.rearrange("(o p) s -> p o s", p=P)
            wqt_r = wqt.rearrange("(o p) m -> p o m", p=P)

            # ---- initial DMAs, ordered so the k chain can start ASAP and
            # q-head weights stream in behind the x chunk.
            rt_sb = cp.tile([P, P], bf)
            nc.sync.dma_start(rt_sb[:], rt[:])
            wk_sb = wp.tile([P, DB, HD], bf)
            nc.sync.dma_start(wk_sb[:], wkt.rearrange("(o p) m -> p o m", p=P))
            xs0 = xtp.tile([P, DB, SC], bf, tag="xs", name="xs0")
            wq_sb = wp.tile([P, DB, DHL], bf)
            for g in range(8):
                nc.sync.dma_start(xs0[:, g * 4:(g + 1) * 4, :],
                                  xt_r[:, g * 4:(g + 1) * 4, 0:SC])
                if g % 2 == 1:
                    gg = g // 2
                    nc.sync.dma_start(wq_sb[:, gg * 8:(gg + 1) * 8, :],
                                      wqt_r[:, gg * 8:(gg + 1) * 8, :])
            wv_sb = wp.tile([P, DB, HD], bf)
            nc.sync.dma_start(wv_sb[:], wvt.rearrange("(o p) m -> p o m", p=P))
            ce_sb = cp.tile([P, SEQ], bf)
            nc.sync.dma_start(ce_sb[:], cexp[:])
            se_sb = cp.tile([P, SEQ], bf)
            nc.sync.dma_start(se_sb[:], sexp[:])
            id_sb = cp.tile([P, P], bf)
            nc.sync.dma_start(id_sb[:], ident[:])
            md_sb = cp.tile([P, P], bf)
            nc.sync.dma_start(md_sb[:], md01[:])
            mt_sb = cp.tile([P, P], bf)
            nc.sync.dma_start(mt_sb[:], mt01[:])
            wo_sb = wp.tile([P, QH, DIM], bf)
            nc.sync.dma_start(wo_sb[:], wot.rearrange("(o p) m -> p o m", p=P))

            warm = psbig.tile([P, P], f32, tag="big", name="warm")
            for i in range(48):
                nc.tensor.matmul(warm, rt_sb[:], rt_sb[:],
                                 start=(i == 0), stop=(i == 47))

            krot = kvp.tile([P, SEQ], bf)          # kT rope'd [dh, s]
            v_sb = kvp.tile([P, NIB, HD + 1], bf)  # v natural [s, dh] + ones
            nc.gpsimd.memset(v_sb[:, :, HD:], 1.0)

            def emit_outproj(at, ci_src, sb):
                ot = osbp.tile([P, DIM], bf, tag="ot")
                for oc in range(DIM // 512):
                    po = psop.tile([P, 512], f32, tag="po")
                    for h2 in range(QH):
                        nc.tensor.matmul(
                            po, at[:, h2, sb * P:(sb + 1) * P],
                            wo_sb[:, h2, oc * 512:(oc + 1) * 512],
                            start=(h2 == 0), stop=(h2 == QH - 1))
                    nc.scalar.copy(ot[:, oc * 512:(oc + 1) * 512], po)
                    if oc == 3:
                        r0 = ci_src * SC + sb * P
                        nc.sync.dma_start(out[r0:r0 + P, :DIM // 2],
                                          ot[:, :DIM // 2])
                r0 = ci_src * SC + sb * P
                nc.sync.dma_start(out[r0:r0 + P, DIM // 2:],
                                  ot[:, DIM // 2:])

            prev = None
            for ci in range(NSC):
                s0 = ci * SC
                bi0 = ci * BPC
                if ci == 0:
                    xs = xs0
                else:
                    xs = xtp.tile([P, DB, SC], bf, tag="xs")
                    for g in range(2):
                        nc.sync.dma_start(xs[:, g * 16:(g + 1) * 16, :],
                                          xt_r[:, g * 16:(g + 1) * 16,
                                               s0:s0 + SC])
                qrot = qrp.tile([P, QH, SC], bf, tag="qrot")

                # ---- QKV projection chains (k first: its weights land first)
                qsbs = []
                for hb in range(QH + 1):
                    ps = psbig.tile([P, SC], f32, tag="big", name="ps")
                    for db in range(DB):
                        lhsT = (wk_sb[:, db, :] if hb == 0
                                else wq_sb[:, db, (hb - 1) * HD:hb * HD])
                        nc.tensor.matmul(ps, lhsT, xs[:, db, :],
                                         start=(db == 0), stop=(db == DB - 1))
                    qsb = t12p.tile([P, SC], bf, tag=f"qsb{hb}")
                    nc.vector.tensor_copy(qsb[:], ps)
                    qsbs.append(qsb)

                # ---- RoPE rotate matmuls (PE) + elementwise (DVE)
                for hb in range(QH + 1):
                    qsb = qsbs[hb]
                    # alternate pools: a 4-deep pr rotation so consecutive
                    # rotate-matmuls never wait on the t2 multiply
                    prp = psbig if hb % 2 == 0 else psop
                    pr = prp.tile([P, SC], f32,
                                  tag="big" if hb % 2 == 0 else "po",
                                  name="pr")
                    nc.tensor.matmul(pr, rt_sb[:], qsb[:], start=True, stop=True)
                    t2 = t12p.tile([P, SC], bf, tag="t2")
                    nc.vector.tensor_mul(t2[:], pr, se_sb[:, s0:s0 + SC])
                    t1 = t12p.tile([P, SC], bf, tag="t1")
                    nc.vector.tensor_mul(t1[:], qsb[:], ce_sb[:, s0:s0 + SC])
                    dst = (krot[:, s0:s0 + SC] if hb == 0
                           else qrot[:, hb - 1, :])
                    nc.vector.tensor_add(dst, t1[:], t2[:])

                # ---- V projection (natural [s, dh]); PE filler during rope DVE
                for sb2 in range(BPC):
                    pv2 = pssc.tile([P, HD], f32, tag="sc", name="pv2")
                    for db in range(DB):
                        nc.tensor.matmul(pv2, xs[:, db, sb2 * P:(sb2 + 1) * P],
                                         wv_sb[:, db, :],
                                         start=(db == 0), stop=(db == DB - 1))
                    nc.vector.tensor_copy(v_sb[:, bi0 + sb2, :HD], pv2)

                # ---- attention for this chunk
                attnT = atp.tile([P, QH, SC], bf, tag="attnT")
                for h in range(QH):
                    et = {}
                    for bj in range(max(0, bi0 - WB), bi0 + BPC):
                        lo = max(bi0, bj)
                        hi = min(bi0 + BPC - 1, bj + WB)
                        qo0 = (lo - bi0) * P
                        w = (hi - lo + 1) * P
                        sc = pssc.tile([P, SC], f32, tag="sc", name="sc")
                        nc.tensor.matmul(sc[:, :w], krot[:, bj * P:(bj + 1) * P],
                                         qrot[:, h, qo0:qo0 + w],
                                         start=True, stop=True)
                        e = etp.tile([P, SC], bf, tag="et")
                        nc.scalar.activation(e[:, :w], sc[:, :w], Exp)
                        if bj >= bi0:  # diagonal block: causal upper-tri zero
                            io = (bj - lo) * P
                            nc.vector.tensor_mul(e[:, io:io + P],
                                                 e[:, io:io + P], md_sb[:])
                        if bj + WB <= bi0 + BPC - 1:  # tail block of window
                            io = (bj + WB - lo) * P
                            nc.vector.tensor_mul(e[:, io:io + P],
                                                 e[:, io:io + P], mt_sb[:])
                        et[bj] = (e, lo)
                    # previous chunk's out-projection: dense PE work that
                    # hides this head's exp latency on ACT
                    if prev is not None:
                        emit_outproj(prev[0], prev[1], h)
                    # pv + normalize for all four blocks first; transposes
                    # batched at the end so they never wait on the normalize
                    asbs = []
                    for bi in range(bi0, bi0 + BPC):
                        js = list(range(max(0, bi - WB), bi + 1))
                        pvp = pspv.tile([P, HD + 1], f32, tag="pvtr",
                                        name="pvp")
                        for idx, bj in enumerate(js):
                            e, lo = et[bj]
                            io = (bi - lo) * P
                            nc.tensor.matmul(pvp, e[:, io:io + P],
                                             v_sb[:, bj, :],
                                             start=(idx == 0),
                                             stop=(idx == len(js) - 1))
                        rec = asp.tile([P, 1], f32, tag="rec")
                        nc.vector.reciprocal(rec[:], pvp[:, HD:HD + 1])
                        asb = asp.tile([P, HD], bf, tag="asb")
                        nc.vector.tensor_scalar_mul(asb[:], pvp[:, :HD],
                                                    rec[:])
                        asbs.append(asb)
                    for k, bi in enumerate(range(bi0, bi0 + BPC)):
                        pt = pspv.tile([P, P], bf, tag="pvtr", name="pt")
                        nc.tensor.transpose(pt[:], asbs[k][:], id_sb[:])
                        nc.vector.tensor_copy(attnT[:, h, (bi - bi0) * P:
                                              (bi - bi0 + 1) * P], pt[:])

                prev = (attnT, ci)

            # drain: out-projection of the final chunk
            for sb in range(BPC):
                emit_outproj(prev[0], prev[1], sb)
    if not nc.is_finalized():
        nc.finalize()
    return nc


def _prep_inputs(x, wq, wk, wv, wo, cos, sin):
    scale = HD ** -0.5
    xtb = np.ascontiguousarray(x.T).astype(BF)
    ce = np.repeat(cos.T, 2, axis=0).astype(BF)          # [128, SEQ]
    se = np.repeat(sin.T, 2, axis=0).astype(BF)
    rtm = np.zeros((P, P), np.float32)
    for i in range(P // 2):
        rtm[2 * i, 2 * i + 1] = 1.0
        rtm[2 * i + 1, 2 * i] = -1.0
    rtm = rtm.astype(BF)
    idm = np.eye(P, dtype=np.float32).astype(BF)
    pp, ff = np.arange(P)[:, None], np.arange(P)[None, :]
    md = (pp <= ff).astype(np.float32).astype(BF)   # diag: keep k <= q
    mt = (ff < pp).astype(np.float32).astype(BF)    # tail: keep q < k

    in_maps = []
    for c in range(NCORES):
        qs, ks = slice(c * DHL, (c + 1) * DHL), slice(c * HD, (c + 1) * HD)
        in_maps.append({
            "xt": xtb,
            "wqt": np.ascontiguousarray((wq[qs] * scale).T).astype(BF),
            "wkt": np.ascontiguousarray(wk[ks].T).astype(BF),
            "wvt": np.ascontiguousarray(wv[ks].T).astype(BF),
            "wot": np.ascontiguousarray(wo[:, qs].T).astype(BF),
            "cexp": ce, "sexp": se, "rt": rtm, "ident": idm,
            "md01": md, "mt01": mt,
        })
    return in_maps


_NC_CACHE = {}


def kernel(x, wq, wk, wv, wo, cos, sin):
    from concourse.bass_utils import run_bass_kernel_spmd

    x = np.asarray(x, np.float32)
    wq = np.asarray(wq, np.float32)
    wk = np.asarray(wk, np.float32)
    wv = np.asarray(wv, np.float32)
    wo = np.asarray(wo, np.float32)
    cos = np.asarray(cos, np.float32)
    sin = np.asarray(sin, np.float32)

    if "nc" not in _NC_CACHE:
        _NC_CACHE["nc"] = _build_nc()
    nc = _NC_CACHE["nc"]
    in_maps = _prep_inputs(x, wq, wk, wv, wo, cos, sin)

    trace = os.environ.get("KERNEL_TRACE", "0") == "1"
    res = None
    if trace:
        try:
            res = run_bass_kernel_spmd(nc, in_maps,
                                       core_ids=list(range(NCORES)),
                                       trace=True)
        except Exception as e:  # profiling hooks absent in some containers
            print(f"trace unavailable ({type(e).__name__}: {e}); "
                  "running untraced")
            res = None
    if res is None:
        res = run_bass_kernel_spmd(nc, in_maps, core_ids=list(range(NCORES)))
    if res.exec_time_ns is not None:
        print(f"HW exec time: {res.exec_time_ns} ns")
    acc = np.zeros((SEQ, DIM), np.float32)
    for c in range(NCORES):
        acc += res.results[c]["out"].astype(np.float32)
    return acc


# revision 30
# speedup vs baseline: 1.0055x; 1.0055x over previous
"""Sliding-window GQA attention on 8 TRN2 NeuronCores, tensor-parallel by heads.

Core c owns KV head c and Q heads 4c..4c+3.  All device matmuls run in bf16.
Structure: 4 sequence chunks of 512; per chunk QKV projection + RoPE, then
windowed attention (scores transposed [k,q], exp on ACT, post-exp 0/1 masks on
DVE), pv with a ones-column denominator, per-partition normalize, PE transpose
to [dh,s], then the wo out-projection.  Chunks pipeline: attention of chunk i
overlaps QKV of chunk i+1 on complementary engines.  Each core emits a partial
output (wo input-dim sharded); the host sums the 8 partials.
"""

import os
import sys

sys.path.insert(0, "/opt/trn_rl_repo")

import numpy as np
import ml_dtypes

SEQ = 2048
DIM = 4096
N_HEADS = 32
N_KV = 8
HD = 128
WIN = 1024
NCORES = 8
QH = N_HEADS // N_KV          # 4 q heads per core
DHL = QH * HD                 # 512 local q dims
P = 128
DB = DIM // P                 # 32 contraction blocks
SC = 512                      # seq chunk
NSC = SEQ // SC               # 4 chunks
BPC = SC // P                 # 4 i-blocks per chunk
NIB = SEQ // P                # 16 blocks total
WB = WIN // P                 # 8 window blocks

BF = ml_dtypes.bfloat16


def _build_nc():
    import concourse.mybir as mybir
    from concourse import bacc
    from concourse.tile import TileContext

    f32 = mybir.dt.float32
    bf = mybir.dt.bfloat16

    nc = bacc.Bacc()
    xt = nc.declare_dram_parameter("xt", [DIM, SEQ], bf, isOutput=False)
    wqt = nc.declare_dram_parameter("wqt", [DIM, DHL], bf, isOutput=False)
    wkt = nc.declare_dram_parameter("wkt", [DIM, HD], bf, isOutput=False)
    wvt = nc.declare_dram_parameter("wvt", [DIM, HD], bf, isOutput=False)
    wot = nc.declare_dram_parameter("wot", [DHL, DIM], bf, isOutput=False)
    cexp = nc.declare_dram_parameter("cexp", [P, SEQ], bf, isOutput=False)
    sexp = nc.declare_dram_parameter("sexp", [P, SEQ], bf, isOutput=False)
    rt = nc.declare_dram_parameter("rt", [P, P], bf, isOutput=False)
    ident = nc.declare_dram_parameter("ident", [P, P], bf, isOutput=False)
    md01 = nc.declare_dram_parameter("md01", [P, P], bf, isOutput=False)
    mt01 = nc.declare_dram_parameter("mt01", [P, P], bf, isOutput=False)
    out = nc.declare_dram_parameter("out", [SEQ, DIM], bf, isOutput=True)

    Exp = mybir.ActivationFunctionType.Exp

    with TileContext(nc) as tc:
        with (
            tc.tile_pool(name="const", bufs=1) as cp,
            tc.tile_pool(name="wp", bufs=1) as wp,
            tc.tile_pool(name="kvp", bufs=1) as kvp,
            tc.tile_pool(name="xtp", bufs=1) as xtp,
            tc.tile_pool(name="qrp", bufs=2) as qrp,
            tc.tile_pool(name="atp", bufs=2) as atp,
            tc.tile_pool(name="t12", bufs=2) as t12p,
            tc.tile_pool(name="expt", bufs=13) as etp,
            tc.tile_pool(name="asp", bufs=5) as asp,
            tc.tile_pool(name="osb", bufs=2) as osbp,
            tc.tile_pool(name="psb", bufs=2, space="PSUM") as psbig,
            tc.tile_pool(name="pss", bufs=2, space="PSUM") as pssc,
            tc.tile_pool(name="pvt", bufs=2, space="PSUM") as pspv,
            tc.tile_pool(name="pso", bufs=2, space="PSUM") as psop,
        ):
            xt_r = xt.rearrange("(o p) s -> p o s", p=P)
            wqt_r = wqt.rearrange("(o p) m -> p o m", p=P)

            # ---- initial DMAs, ordered so the k chain can start ASAP and
            # q-head weights stream in behind the x chunk.
            wk_sb = wp.tile([P, DB, HD], bf)
            nc.sync.dma_start(wk_sb[:], wkt.rearrange("(o p) m -> p o m", p=P))
            xs0 = xtp.tile([P, DB, SC], bf, tag="xs", name="xs0")
            wq_sb = wp.tile([P, DB, DHL], bf)
            for g in range(8):
                nc.sync.dma_start(xs0[:, g * 4:(g + 1) * 4, :],
                                  xt_r[:, g * 4:(g + 1) * 4, 0:SC])
                if g % 2 == 1:
                    gg = g // 2
                    nc.sync.dma_start(wq_sb[:, gg * 8:(gg + 1) * 8, :],
                                      wqt_r[:, gg * 8:(gg + 1) * 8, :])
            wv_sb = wp.tile([P, DB, HD], bf)
            nc.sync.dma_start(wv_sb[:], wvt.rearrange("(o p) m -> p o m", p=P))
            ce_sb = cp.tile([P, SEQ], bf)
            nc.sync.dma_start(ce_sb[:], cexp[:])
            se_sb = cp.tile([P, SEQ], bf)
            nc.sync.dma_start(se_sb[:], sexp[:])
            rt_sb = cp.tile([P, P], bf)
            nc.sync.dma_start(rt_sb[:], rt[:])
            id_sb = cp.tile([P, P], bf)
            nc.sync.dma_start(id_sb[:], ident[:])
            md_sb = cp.tile([P, P], bf)
            nc.sync.dma_start(md_sb[:], md01[:])
            mt_sb = cp.tile([P, P], bf)
            nc.sync.dma_start(mt_sb[:], mt01[:])
            wo_sb = wp.tile([P, QH, DIM], bf)
            nc.sync.dma_start(wo_sb[:], wot.rearrange("(o p) m -> p o m", p=P))

            krot = kvp.tile([P, SEQ], bf)          # kT rope'd [dh, s]
            v_sb = kvp.tile([P, NIB, HD + 1], bf)  # v natural [s, dh] + ones
            nc.gpsimd.memset(v_sb[:, :, HD:], 1.0)

            def emit_outproj(at, ci_src, sb):
                ot = osbp.tile([P, DIM], bf, tag="ot")
                for oc in range(DIM // 512):
                    po = psop.tile([P, 512], f32, tag="po")
                    for h2 in range(QH):
                        nc.tensor.matmul(
                            po, at[:, h2, sb * P:(sb + 1) * P],
                            wo_sb[:, h2, oc * 512:(oc + 1) * 512],
                            start=(h2 == 0), stop=(h2 == QH - 1))
                    nc.scalar.copy(ot[:, oc * 512:(oc + 1) * 512], po)
                    if oc == 3:
                        r0 = ci_src * SC + sb * P
                        nc.sync.dma_start(out[r0:r0 + P, :DIM // 2],
                                          ot[:, :DIM // 2])
                r0 = ci_src * SC + sb * P
                nc.sync.dma_start(out[r0:r0 + P, DIM // 2:],
                                  ot[:, DIM // 2:])

            prev = None
            for ci in range(NSC):
                s0 = ci * SC
                bi0 = ci * BPC
                if ci == 0:
                    xs = xs0
                else:
                    xs = xtp.tile([P, DB, SC], bf, tag="xs")
                    for g in range(4):
                        nc.sync.dma_start(xs[:, g * 8:(g + 1) * 8, :],
                                          xt_r[:, g * 8:(g + 1) * 8, s0:s0 + SC])
                qrot = qrp.tile([P, QH, SC], bf, tag="qrot")

                # ---- QKV projection chains (k first: its weights land first)
                qsbs = []
                for hb in range(QH + 1):
                    ps = psbig.tile([P, SC], f32, tag="big", name="ps")
                    for db in range(DB):
                        lhsT = (wk_sb[:, db, :] if hb == 0
                                else wq_sb[:, db, (hb - 1) * HD:hb * HD])
                        nc.tensor.matmul(ps, lhsT, xs[:, db, :],
                                         start=(db == 0), stop=(db == DB - 1))
                    qsb = t12p.tile([P, SC], bf, tag=f"qsb{hb}")
                    nc.vector.tensor_copy(qsb[:], ps)
                    qsbs.append(qsb)

                # ---- RoPE rotate matmuls (PE) + elementwise (DVE)
                for hb in range(QH + 1):
                    qsb = qsbs[hb]
                    # alternate pools: a 4-deep pr rotation so consecutive
                    # rotate-matmuls never wait on the t2 multiply
                    prp = psbig if hb % 2 == 0 else psop
                    pr = prp.tile([P, SC], f32,
                                  tag="big" if hb % 2 == 0 else "po",
                                  name="pr")
                    nc.tensor.matmul(pr, rt_sb[:], qsb[:], start=True, stop=True)
                    t2 = t12p.tile([P, SC], bf, tag="t2")
                    nc.vector.tensor_mul(t2[:], pr, se_sb[:, s0:s0 + SC])
                    t1 = t12p.tile([P, SC], bf, tag="t1")
                    nc.vector.tensor_mul(t1[:], qsb[:], ce_sb[:, s0:s0 + SC])
                    dst = (krot[:, s0:s0 + SC] if hb == 0
                           else qrot[:, hb - 1, :])
                    nc.vector.tensor_add(dst, t1[:], t2[:])

                # ---- V projection (natural [s, dh]); PE filler during rope DVE
                for sb2 in range(BPC):
                    pv2 = pssc.tile([P, HD], f32, tag="sc", name="pv2")
                    for db in range(DB):
                        nc.tensor.matmul(pv2, xs[:, db, sb2 * P:(sb2 + 1) * P],
                                         wv_sb[:, db, :],
                                         start=(db == 0), stop=(db == DB - 1))
                    nc.vector.tensor_copy(v_sb[:, bi0 + sb2, :HD], pv2)

                # ---- attention for this chunk
                attnT = atp.tile([P, QH, SC], bf, tag="attnT")
                for h in range(QH):
                    et = {}
                    for bj in range(max(0, bi0 - WB), bi0 + BPC):
                        lo = max(bi0, bj)
                        hi = min(bi0 + BPC - 1, bj + WB)
                        qo0 = (lo - bi0) * P
                        w = (hi - lo + 1) * P
                        sc = pssc.tile([P, SC], f32, tag="sc", name="sc")
                        nc.tensor.matmul(sc[:, :w], krot[:, bj * P:(bj + 1) * P],
                                         qrot[:, h, qo0:qo0 + w],
                                         start=True, stop=True)
                        e = etp.tile([P, SC], bf, tag="et")
                        nc.scalar.activation(e[:, :w], sc[:, :w], Exp)
                        if bj >= bi0:  # diagonal block: causal upper-tri zero
                            io = (bj - lo) * P
                            nc.vector.tensor_mul(e[:, io:io + P],
                                                 e[:, io:io + P], md_sb[:])
                        if bj + WB <= bi0 + BPC - 1:  # tail block of window
                            io = (bj + WB - lo) * P
                            nc.vector.tensor_mul(e[:, io:io + P],
                                                 e[:, io:io + P], mt_sb[:])
                        et[bj] = (e, lo)
                    # previous chunk's out-projection: dense PE work that
                    # hides this head's exp latency on ACT
                    if prev is not None:
                        emit_outproj(prev[0], prev[1], h)
                    # pv + normalize for all four blocks first; transposes
                    # batched at the end so they never wait on the normalize
                    asbs = []
                    for bi in range(bi0, bi0 + BPC):
                        js = list(range(max(0, bi - WB), bi + 1))
                        pvp = pspv.tile([P, HD + 1], f32, tag="pvtr",
                                        name="pvp")
                        for idx, bj in enumerate(js):
                            e, lo = et[bj]
                            io = (bi - lo) * P
                            nc.tensor.matmul(pvp, e[:, io:io + P],
                                             v_sb[:, bj, :],
                                             start=(idx == 0),
                                             stop=(idx == len(js) - 1))
                        rec = asp.tile([P, 1], f32, tag="rec")
                        nc.vector.reciprocal(rec[:], pvp[:, HD:HD + 1])
                        asb = asp.tile([P, HD], bf, tag="asb")
                        nc.vector.tensor_scalar_mul(asb[:], pvp[:, :HD],
                                                    rec[:])
                        asbs.append(asb)
                    for k, bi in enumerate(range(bi0, bi0 + BPC)):
                        pt = pspv.tile([P, P], bf, tag="pvtr", name="pt")
                        nc.tensor.transpose(pt[:], asbs[k][:], id_sb[:])
                        nc.vector.tensor_copy(attnT[:, h, (bi - bi0) * P:
                                              (bi - bi0 + 1) * P], pt[:])

                prev = (attnT, ci)

            # drain: out-projection of the final chunk
            for sb in range(BPC):
                emit_outproj(prev[0], prev[1], sb)
    if not nc.is_finalized():
        nc.finalize()
    return nc


def _prep_inputs(x, wq, wk, wv, wo, cos, sin):
    scale = HD ** -0.5
    xtb = np.ascontiguousarray(x.T).astype(BF)
    ce = np.repeat(cos.T, 2, axis=0).astype(BF)          # [128, SEQ]
    se = np.repeat(sin.T, 2, axis=0).astype(BF)
    rtm = np.zeros((P, P), np.float32)
    for i in range(P // 2):
        rtm[2 * i, 2 * i + 1] = 1.0
        rtm[2 * i + 1, 2 * i] = -1.0
    rtm = rtm.astype(BF)
    idm = np.eye(P, dtype=np.float32).astype(BF)
    pp, ff = np.arange(P)[:, None], np.arange(P)[None, :]
    md = (pp <= ff).astype(np.float32).astype(BF)   # diag: keep k <= q
    mt = (ff < pp).astype(np.float32).astype(BF)    # tail: keep q < k

    in_maps = []
    for c in range(NCORES):
        qs, ks = slice(c * DHL, (c + 1) * DHL), slice(c * HD, (c + 1) * HD)
        in_maps.append({
            "xt": xtb,
            "wqt": np.ascontiguousarray((wq[qs] * scale).T).astype(BF),
            "wkt": np.ascontiguousarray(wk[ks].T).astype(BF),
            "wvt": np.ascontiguousarray(wv[ks].T).astype(BF),
            "wot": np.ascontiguousarray(wo[:, qs].T).astype(BF),
            "cexp": ce, "sexp": se, "rt": rtm, "ident": idm,
            "md01": md, "mt01": mt,
        })
    return in_maps


_NC_CACHE = {}


def kernel(x, wq, wk, wv, wo, cos, sin):
    from concourse.bass_utils import run_bass_kernel_spmd

    x = np.asarray(x, np.float32)
    wq = np.asarray(wq, np.float32)
    wk = np.asarray(wk, np.float32)
    wv = np.asarray(wv, np.float32)
    wo = np.asarray(wo, np.float32)
    cos = np.asarray(cos, np.float32)
    sin = np.asarray(sin, np.float32)

    if "nc" not in _NC_CACHE:
        _NC_CACHE["nc"] = _build_nc()
    nc = _NC_CACHE["nc"]
    in_maps = _prep_inputs(x, wq, wk, wv, wo, cos, sin)

    trace = os.environ.get("KERNEL_TRACE", "0") == "1"
    res = None
    if trace:
        try:
            res = run_bass_kernel_spmd(nc, in_maps,
                                       core_ids=list(range(NCORES)),
                                       trace=True)
        except Exception as e:  # profiling hooks absent in some containers
            print(f"trace unavailable ({type(e).__name__}: {e}); "
                  "running untraced")
            res = None
    if res is None:
        res = run_bass_kernel_spmd(nc, in_maps, core_ids=list(range(NCORES)))
    if res.exec_time_ns is not None:
        print(f"HW exec time: {res.exec_time_ns} ns")
    acc = np.zeros((SEQ, DIM), np.float32)
    for c in range(NCORES):
        acc += res.results[c]["out"].astype(np.float32)
    return acc
